# revision 36
# baseline (speedup 1.0000x reference)
"""ChebyKAN linear layer on 8 Trainium2 NeuronCores.

Math: y[b,o] = sum_{i,d} T_d(w[b,i]) * C[i,o,d], with w = tanh(tanh(x)) and
T_d the Chebyshev polynomials (cos(d*arccos(w)) == T_d(w) for |w|<=1).

The ACT engine has no arccos/cos, so the device evaluates the Chebyshev-product
basis phi = [T1, T1^2, T1*T2, T2^2, T2*T3, T3^2, T3*T4, T4^2] built from
Square/multiply ops (T2, T4 and the T3 helper come from cheap affine ops). Via
T_{2k} = 2*T_k^2-1 and T_{m+n} = 2*T_m*T_n - T_{m-n}, an exact host-side
linear transform maps Chebyshev coefficients onto this basis with O(1)
conditioning; the constant column folds into a per-o bias added during PSUM
evacuation (shipped x16; the host divides the gathered output by 16, exact in
fp32).

Sharding: data-parallel over batch b (16384 -> 2048/core); coeffs replicated.
x is pre-transposed on the host so the contraction dim (c_in) lands on SBUF
partitions; the kernel computes y^T per core and the host transposes back.

Everything on device is fp16: x ships as fp16 (2MB/core), weights as fp16
scaled x16 on the host (4MB/core; the raw values ~1e-4 would graze the fp16
subnormal floor), and the whole basis chain computes in fp16 (10 mantissa
bits -> measured end-to-end error ~1.2e-3 vs the 2e-2 gate). fp16 matmuls
stream 1 cycle/row -- measured 213ns per 512-row matmul, the PE floor --
PSUM accumulates fp32, and the fp16-native chain needs no cast ops: ACT runs
the critical path (tanh -> tanh -> Square -> affine, 4 ops/row-block), DVE
runs the 9 off-critical multiply/affine ops at 16-bit double rate.

Batch schedule: four uniform 512-wide PSUM phases. (Narrow 256-wide head
phases were tried to start the stream ~1.5us earlier, but they consume the
row-block-0 weight chunks at 2x the rate -- ~300GB/s demanded exactly during
the DMA ramp, which only delivers ~200GB/s -- so LDWEIGHTS stalled longer
than the narrow head saved, and every such stall also drops the PE HAM clock
gate.) A 128KB x-sliver for the first row-block still ships ahead of
everything so the tanh chain and first matmul start ~10.9us.

Scheduling lessons baked in (from NTFF profiles of prior revisions): the
engine queues are strictly in-order, so EMISSION order is the schedule.
(1) PSUM evacuations for a phase are emitted after the NEXT phase's first
row-block: any earlier they head-of-line block the ACT/DVE queues waiting
for the accumulation group to stop (~2-3us PE stall per phase boundary,
which also drops the PE HAM clock gate 2.4->1.2GHz and bleeds another ~3us);
the banks are not reused until one more phase later (bufs=2), so late
evacuation is free. Evacs alternate ACT/DVE (bias via the per-partition AP
scalar of tensor_scalar). (2) All input DMAs ride the sync HWDGE ring in
exact consumption order -- the 16 DMA engines round-robin ACTIVE transfers,
so issuing a late-needed 1MB load early steals bandwidth from the critical
first tiles (measured: gpsimd SWDGE delivers only ~110GB/s per queue and
serialized the row-block-0 weight chunks, starving LDWEIGHTS ~10us).
(3) gpsimd (Q7) gets no elementwise work (measured ~1.5us per [128,512]
cast, 2-3x the cost model) and no PSUM access; it only issues the hidden
mid-phase output DMAs. The final phase's outputs use the two fast HWDGE
rings (sync/scalar, ~360GB/s). (4) A ~3.4us burst of fp32 dummy matmuls
warms the PE HAM clock gate during the DMA ramp; a tiny bias load goes first
on the gpsimd ring to absorb the ~1.5us DMA-engine doorbell-to-packet wake.
(5) tile_wait_until stamps per row-block bias the Tile scheduler toward
consumption order (its DMA-completion predictions are optimistic).
"""

import sys

if "/opt/trn_rl_repo" not in sys.path:
    sys.path.append("/opt/trn_rl_repo")

import numpy as np

import concourse.bacc as bacc
import concourse.tile as tile
from concourse import mybir
from concourse.bass_utils import run_bass_kernel_spmd

DEGREE = 8
B, C_IN, C_OUT = 16384, 512, 512
N_CORES = 8
NB = B // N_CORES            # 2048 batch rows per core
N_IB = C_IN // 128           # 4 contraction row-blocks
N_J = DEGREE                 # basis funcs phi_1..phi_8 (constant -> bias)
F32 = mybir.dt.float32
F16 = mybir.dt.float16
W_SCALE = 16.0               # host premultiplies weights+bias; host divides y

WIDE = [(0, 512), (512, 512), (1024, 512), (1536, 512)]
assert sum(w for _, w in WIDE) == NB

_CACHE = {}


def _build():
    nc = bacc.Bacc("TRN2", target_bir_lowering=False, debug=False)
    # x ships pre-packed on the host into consumption order: one contiguous
    # [128, width] segment per (phase, row-block) so every x load is a single
    # DMA with fat (3.5-4KB) descriptors -- 256-column slices of the natural
    # [c_in, b] layout produce 512B descriptors and the early DMA window
    # measured only ~175GB/s, landing weights late.
    xt = nc.dram_tensor("xt", [128, NB * N_IB], F16, kind="ExternalInput")
    wmat = nc.dram_tensor("wmat", [C_IN, N_J * C_OUT], F16, kind="ExternalInput")
    biasv = nc.dram_tensor("biasv", [128, 4], F32, kind="ExternalInput")
    yt = nc.dram_tensor("yt", [C_OUT, NB], F32, kind="ExternalOutput")

    Tanh = mybir.ActivationFunctionType.Tanh
    Square = mybir.ActivationFunctionType.Square
    Identity = mybir.ActivationFunctionType.Identity
    ALU_MULT = mybir.AluOpType.mult
    ALU_ADD = mybir.AluOpType.add

    with tile.TileContext(nc) as tc:
        with (
            tc.tile_pool(name="const", bufs=1) as const_pool,
            tc.tile_pool(name="wts", bufs=1) as wpool,
            tc.tile_pool(name="pows", bufs=2) as ppool,
            tc.tile_pool(name="outs", bufs=2) as opool,
            tc.tile_pool(name="psum", bufs=2, space="PSUM") as pspool,
        ):
            # PE warm-up fodder while the first DMAs are in flight.
            dummy = const_pool.tile([128, 128], F32, tag="dummy")
            nc.gpsimd.memset(dummy[:], 0.0)
            cm1 = const_pool.tile([128, 1], F16, tag="cm1")
            nc.gpsimd.memset(cm1[:], -1.0)
            dps = pspool.tile([128, 512], F32, tag="ps3", name="dps")
            for _ in range(10):
                nc.tensor.matmul(
                    dps[:, 0:128], lhsT=dummy[:], rhs=dummy[:],
                    start=True, stop=True,
                )

            # Tiny bias load first on gpsimd: absorbs the DMA-engine wake.
            bias_t = const_pool.tile([128, 4], F32)
            nc.gpsimd.dma_start(out=bias_t[:], in_=biasv.ap())

            # All input loads on the sync HWDGE ring in consumption order.
            w_sb = [
                wpool.tile([128, N_J * C_OUT], F16, tag=f"wc{ib}", name=f"wc{ib}")
                for ib in range(N_IB)
            ]

            def load_w(ib, j0, j1):
                nc.sync.dma_start(
                    out=w_sb[ib][:, j0 * C_OUT : j1 * C_OUT],
                    in_=wmat.ap()[ib * 128 : (ib + 1) * 128, j0 * C_OUT : j1 * C_OUT],
                )

            # SBUF x tiles mirror the packed DRAM layout: one [128, 2048]
            # tile per 512-wide phase (4 x 512-col row-block segments).
            xw_t = [
                ppool.tile([128, 2048], F16, tag=f"xw{pi}", bufs=1, name=f"xw{pi}")
                for pi in range(len(WIDE))
            ]
            # 128KB (ph0, ib0) sliver gates the tanh chain -> first matmul;
            # then the row-block-0 weight chunks race the PE's j-consumption
            # (one 128KB chunk per 852ns of streaming); bulk x and later
            # weight blocks ride behind with ~2us slack each.
            nc.sync.dma_start(out=xw_t[0][:, 0:512], in_=xt.ap()[:, 0:512])
            # Drain the ring so the sliver flies alone at full bandwidth
            # (concurrent transfers round-robin and would delay it ~1.5us);
            # it also absorbs the DMA-engine wake for everything behind.
            nc.sync.drain()
            load_w(0, 0, 1)
            load_w(0, 1, 2)
            nc.sync.dma_start(out=xw_t[0][:, 512:1024], in_=xt.ap()[:, 512:1024])
            load_w(0, 2, 3)
            load_w(0, 3, 4)
            nc.sync.dma_start(out=xw_t[0][:, 1024:2048], in_=xt.ap()[:, 1024:2048])
            load_w(0, 4, 5)
            load_w(0, 5, 6)
            load_w(0, 6, 7)
            load_w(0, 7, 8)
            load_w(1, 0, 4)
            load_w(1, 4, 8)
            load_w(2, 0, 8)
            nc.sync.dma_start(out=xw_t[1][:], in_=xt.ap()[:, 2048:4096])
            load_w(3, 0, 8)
            nc.sync.dma_start(out=xw_t[2][:], in_=xt.ap()[:, 4096:6144])
            nc.sync.dma_start(out=xw_t[3][:], in_=xt.ap()[:, 6144:8192])

            def w_chunk(ib, j, oc):
                return w_sb[ib][:, j * C_OUT + oc * 128 : j * C_OUT + (oc + 1) * 128]

            def emit_chain_and_mms(ps, xsl, ib, wd, start_ib, stop_ib,
                                   final_evac=None):
                # fp16 Chebyshev-product basis chain. Critical path
                # (tanh -> tanh -> Square -> affine) on ACT; DVE runs the
                # off-critical ops at 16-bit 2x rate.
                s = slice(0, wd)
                t1 = ppool.tile([128, 512], F16, tag="t1", bufs=4)
                f2 = ppool.tile([128, 512], F16, tag="f2", bufs=4)
                t2 = ppool.tile([128, 512], F16, tag="t2", bufs=4)
                u3 = ppool.tile([128, 512], F16, tag="u3", bufs=4)
                t3 = ppool.tile([128, 512], F16, tag="t3", bufs=4)
                f3 = ppool.tile([128, 512], F16, tag="f3", bufs=4)
                f4 = ppool.tile([128, 512], F16, tag="f4", bufs=4)
                t4 = ppool.tile([128, 512], F16, tag="t4", bufs=4)
                f5 = ppool.tile([128, 512], F16, tag="f5", bufs=4)
                f6 = ppool.tile([128, 512], F16, tag="f6", bufs=4)
                f7 = ppool.tile([128, 512], F16, tag="f7", bufs=4)
                f8 = ppool.tile([128, 512], F16, tag="f8", bufs=4)
                nc.scalar.activation(xsl, xsl, Tanh)
                nc.scalar.activation(t1[:, s], xsl, Tanh)
                nc.scalar.activation(f2[:, s], t1[:, s], Square)
                nc.scalar.activation(
                    t2[:, s], f2[:, s], Identity, bias=cm1[:], scale=2.0
                )
                nc.vector.tensor_scalar(
                    u3[:, s], f2[:, s], 4.0, -3.0, ALU_MULT, ALU_ADD
                )
                nc.vector.tensor_mul(t3[:, s], t1[:, s], u3[:, s])
                nc.vector.tensor_mul(f3[:, s], t1[:, s], t2[:, s])
                nc.vector.tensor_mul(f4[:, s], t2[:, s], t2[:, s])
                nc.vector.tensor_scalar(
                    t4[:, s], f4[:, s], 2.0, -1.0, ALU_MULT, ALU_ADD
                )
                nc.vector.tensor_mul(f5[:, s], t2[:, s], t3[:, s])
                nc.vector.tensor_mul(f6[:, s], t3[:, s], t3[:, s])
                nc.vector.tensor_mul(f7[:, s], t3[:, s], t4[:, s])
                nc.vector.tensor_mul(f8[:, s], t4[:, s], t4[:, s])
                chunks = [t1, f2, f3, f4, f5, f6, f7, f8]
                if ib < N_IB - 1:
                    order = [(j, oc) for j in range(N_J) for oc in range(4)]
                else:
                    # oc-major on the last row-block: groups finish staggered
                    order = [(j, oc) for oc in range(4) for j in range(N_J)]
                for j, oc in order:
                    nc.tensor.matmul(
                        ps[oc][:, s],
                        lhsT=w_chunk(ib, j, oc),
                        rhs=chunks[j][:, s],
                        start=(ib == start_ib and j == 0),
                        stop=(ib == stop_ib and j == N_J - 1),
                    )
                    if final_evac is not None and ib == stop_ib and j == N_J - 1:
                        # final phase: evacuate each oc group right after its
                        # stop matmul so the drain overlaps the staggered
                        # oc-major tail instead of queuing behind the whole
                        # chain.
                        emit_one_evac(*final_evac, oc)

            def emit_one_evac(ps, off, wd, final, oc):
                s = slice(0, wd)
                osb = opool.tile([128, 512], F32, tag=f"osb{oc}", name=f"osb{oc}")
                if oc % 2 == 0:
                    nc.scalar.activation(
                        osb[:, s], ps[oc][:, s], Identity,
                        bias=bias_t[:, oc : oc + 1],
                    )
                else:
                    nc.vector.tensor_scalar(
                        osb[:, s], ps[oc][:, s], bias_t[:, oc : oc + 1],
                        None, ALU_ADD,
                    )
                if final:
                    out_eng = (nc.scalar, nc.sync, nc.scalar, nc.sync)[oc]
                else:
                    out_eng = nc.gpsimd
                out_eng.dma_start(
                    out=yt.ap()[oc * 128 : (oc + 1) * 128, off : off + wd],
                    in_=osb[:, s],
                )

            def emit_evacs(ps, off, wd, final):
                # Evacuate a completed phase's PSUM banks; called after the
                # NEXT phase's first row-block so the in-order ACT/DVE
                # queues never wait on a still-accumulating group.
                for oc in range(4):
                    emit_one_evac(ps, off, wd, final, oc)

            # Virtual-time stamps (ms): when each row-block's matmuls can
            # start (10.5us prologue + PE time so far).
            t_ms = 0.0105
            pending = []
            for pi, (off, wd) in enumerate(WIDE):
                ps = [
                    pspool.tile([128, 512], F32, tag=f"ps{oc}", name=f"ps{oc}_w{pi}")
                    for oc in range(4)
                ]
                final = pi == len(WIDE) - 1
                for ib in range(N_IB):
                    with tc.tile_wait_until(t_ms):
                        emit_chain_and_mms(
                            ps, xw_t[pi][:, ib * 512 : (ib + 1) * 512],
                            ib, wd, 0, N_IB - 1,
                            final_evac=(ps, off, wd, True) if final else None,
                        )
                        if ib == 0:
                            for pps, poff, pwd in pending:
                                emit_evacs(pps, poff, pwd, False)
                            pending = []
                    t_ms += wd * 128 * 0.4167 * 1e-6
                if not final:
                    pending = [(ps, off, wd)]
    nc.compile()
    return nc


def _host_transform(cheby_coeffs):
    # Map Chebyshev coefficients onto the device phi basis:
    # phi = [T1, T1^2, T1*T2, T2^2, T2*T3, T3^2, T3*T4, T4^2] and a constant.
    # T_{2k} = 2*T_k^2 - 1, T_{m+n} = 2*T_m*T_n - T_{m-n} =>
    #   y = bias + (C1-C3-C5-C7)*T1 + sum_{d=2..8} 2*C_d * phi_{d-1}
    #   bias_o = sum_i (C0 - C2 - C4 - C6 - C8)
    C64 = cheby_coeffs.astype(np.float64)
    bias = (C64[..., 0] - C64[..., 2] - C64[..., 4] - C64[..., 6] - C64[..., 8]).sum(
        axis=0
    )
    W = np.empty((C_IN, C_OUT, N_J), np.float64)
    W[..., 0] = C64[..., 1] - C64[..., 3] - C64[..., 5] - C64[..., 7]
    for d in range(2, DEGREE + 1):
        W[..., d - 1] = 2.0 * C64[..., d]
    # [i, j*512+o]: per-partition-contiguous coefficient rows; fp16 on device,
    # premultiplied by W_SCALE (undone on the host) to clear the fp16
    # subnormal floor. The bias is folded in at the same scale.
    Wd = np.ascontiguousarray(
        (W.transpose(0, 2, 1).reshape(C_IN, N_J * C_OUT) * W_SCALE).astype(np.float16)
    )
    bias_dev = np.ascontiguousarray(
        (bias * W_SCALE).reshape(4, 128).T.astype(np.float32)
    )
    return Wd, bias_dev


def _pack_x(xTc):
    # Repack one core's [c_in, nb] slice into the device's consumption-ordered
    # [128, 8192] layout: one contiguous 512-col segment per (phase,
    # row-block) so each phase loads as a single fat-descriptor DMA.
    xq = xTc.astype(np.float16)
    out = np.empty((128, NB * N_IB), np.float16)
    col = 0
    for off, wd in WIDE:
        for ib in range(N_IB):
            out[:, col : col + wd] = xq[ib * 128 : (ib + 1) * 128, off : off + wd]
            col += wd
    assert col == NB * N_IB
    return np.ascontiguousarray(out)


def make_in_maps(x, cheby_coeffs):
    Wd, bias_dev = _host_transform(cheby_coeffs)
    xT = np.ascontiguousarray(x.T)                       # [c_in, b]
    return [
        {
            "xt": _pack_x(xT[:, c * NB : (c + 1) * NB]),
            "wmat": Wd,
            "biasv": bias_dev,
        }
        for c in range(N_CORES)
    ]


def kernel(x, cheby_coeffs):
    x = np.asarray(x, dtype=np.float32)
    cheby_coeffs = np.asarray(cheby_coeffs, dtype=np.float32)
    if "nc" not in _CACHE:
        _CACHE["nc"] = _build()
    nc = _CACHE["nc"]

    in_maps = make_in_maps(x, cheby_coeffs)
    res = run_bass_kernel_spmd(nc, in_maps, core_ids=list(range(N_CORES)))
    y = np.concatenate([res.results[c]["yt"].T for c in range(N_CORES)], axis=0)
    return (y * np.float32(1.0 / W_SCALE)).astype(np.float32)
